# revision 9
# baseline (speedup 1.0000x reference)
"""Causal self-attention (B=2, T=2048, C=1024, H=16) on 8 Trainium2 cores.

Sharding: data-parallel over batch (2) x tensor-parallel over heads (4 groups
of 4 heads). Core c handles batch b = c//4, head group g = c%4 (heads 4g..4g+3).
Each core computes its qkv column slice, full causal TxT attention for its 4
heads, and a partial row-parallel projection. Host sums the 4 partial proj
outputs per batch and adds b_proj.

v2 design notes (vs the f32r v1):
- everything bf16 end-to-end (inputs, SBUF intermediates, output); PSUM stays
  fp32. Halves DMA traffic (6.5MB in / 4MB out per core) and enables the PE's
  fast-weight-load path (2x LDWEIGHTS for non-fp32 128-col stationaries).
- the scalar engine's exp is the true serializer (~10.5M score exps/core at
  1 elem/lane/cycle): the qkv projection phase is fused INTO the attention
  loop (qkv for t-slice tsl emits interleaved with attention for i-block
  tsl-1) so ACT starts exp'ing ~10us in and stays saturated while the PE
  retires qkv/score/AV/proj matmuls underneath it.
- AV matmuls are column-tiled pairs (head hi=0 -> PE cols 0:64, hi=1 ->
  64:128) running concurrently, each M=64 with no zero padding; the two
  score matmuls of a head pair stay row-tiled (K=64 quadrants) as in v1.
- softmax denominators: DVE accumulates masked exp tiles into two bf16
  chains (even/odd j-chunks); a column-tiled pair of K=128 ones-stationary
  matmuls then both reduces over j and broadcasts den across partitions in
  PSUM rows matching the AV layout; gpsimd drains den to SBUF and one DVE
  divide normalizes into yT. (No ACT Reciprocal: it lives in a different
  ACT table set than Exp and each switch costs ~2.7us.)
- PSUM budget exactly 8 banks: scores [128,2,512]x2 (4) + AV [128,512]x2 (2)
  + one shared round-robin tag for qkv/den/proj psum [128,512]x2 (2).
- proj output copies + DMA and den drains ride the otherwise-idle gpsimd.
"""

import os
import sys

sys.path.insert(0, "/opt/trn_rl_repo")

import numpy as np

P = 128
T = 2048
C = 1024
D = 64
HPC = 4          # heads per core
HD = HPC * D     # 256 qkv columns per core
CC = C // P      # 8 contraction chunks
TC = T // P      # 16 t-chunks of 128
IC = T // 512    # 4 i-chunks of 512

_NC = None
LAST_RESULTS = None


def _build_nc():
    import concourse.mybir as mybir
    import concourse.tile as tile
    from concourse import bacc
    from contextlib import ExitStack

    dt = mybir.dt
    f32 = dt.float32
    bf16 = dt.bfloat16
    ALU = mybir.AluOpType
    ACTF = mybir.ActivationFunctionType

    nc = bacc.Bacc(
        "TRN2",
        target_bir_lowering=False,
        debug=False,
        enable_asserts=False,
        num_devices=8,
    )

    xT = nc.dram_tensor("xT", [C, T], bf16, kind="ExternalInput").ap()
    wq = nc.dram_tensor("wq", [C, HD], bf16, kind="ExternalInput").ap()
    wk = nc.dram_tensor("wk", [C, HD], bf16, kind="ExternalInput").ap()
    wv = nc.dram_tensor("wv", [C, HD], bf16, kind="ExternalInput").ap()
    bq = nc.dram_tensor("bq", [P, 2], f32, kind="ExternalInput").ap()
    bk = nc.dram_tensor("bk", [P, 2], f32, kind="ExternalInput").ap()
    bv = nc.dram_tensor("bv", [P, HD], f32, kind="ExternalInput").ap()
    wp = nc.dram_tensor("wp", [HD, C], bf16, kind="ExternalInput").ap()
    tri = nc.dram_tensor("tri", [P, P], bf16, kind="ExternalInput").ap()
    onesd = nc.dram_tensor("onesd", [P, D], bf16, kind="ExternalInput").ap()
    out = nc.dram_tensor("out", [T, C], bf16, kind="ExternalOutput").ap()

    with tile.TileContext(nc) as tc, ExitStack() as ctx:
        persist = ctx.enter_context(tc.tile_pool(name="persist", bufs=1))
        xT_sb = persist.tile([P, CC, T], bf16, name="xTs")
        wq_sb = persist.tile([P, CC, HD], bf16, name="wqs")
        wk_sb = persist.tile([P, CC, HD], bf16, name="wks")
        wv_sb = persist.tile([P, CC, HD], bf16, name="wvs")
        wp_sb = persist.tile([P, 2, C], bf16, name="wps")
        qT_sb = persist.tile([P, 2, T], bf16, name="qT")    # [d%128, hp, t]
        kT_sb = persist.tile([P, 2, T], bf16, name="kT")
        v_sb = persist.tile([P, TC, 2, 2, D], bf16, name="v")  # [t%128, tc, hp, hi, d]
        yT_sb = persist.tile([P, 2, T], bf16, name="yT")
        tri_sb = persist.tile([P, P], bf16, name="tris")
        ones_sb = persist.tile([P, D], bf16, name="ones")
        bq_sb = persist.tile([P, 2], f32, name="bqs")
        bk_sb = persist.tile([P, 2], f32, name="bks")
        bv_sb = persist.tile([P, 2, 2, D], f32, name="bvs")
        scr_sb = persist.tile([P, 512], bf16, name="scr")

        # ---- input DMA schedule: weights first, x t-slice-major, descriptor
        # generation spread over four engine queues so transfers start fast ----
        xTr = xT.rearrange("(o p) t -> p o t", p=P)
        wqr = wq.rearrange("(o p) n -> p o n", p=P)
        wkr = wk.rearrange("(o p) n -> p o n", p=P)
        nc.vector.memset(scr_sb[:], 0.0)
        nc.sync.dma_start(wq_sb[:, :, 0:P], wqr[:, :, 0:P])
        nc.gpsimd.dma_start(wk_sb[:, :, 0:P], wkr[:, :, 0:P])
        nc.scalar.dma_start(tri_sb[:], tri)
        nc.scalar.dma_start(bq_sb[:], bq)
        nc.scalar.dma_start(bk_sb[:], bk)
        nc.scalar.dma_start(ones_sb[:], onesd)
        nc.scalar.dma_start(
            bv_sb[:], bv.rearrange("p (hp hi d) -> p hp hi d", hi=2, d=D)
        )
        ENGS = (nc.sync, nc.gpsimd, nc.scalar, nc.sync)

        def x_slice(tsl):
            for cc in range(CC):
                ENGS[cc % 4].dma_start(
                    xT_sb[:, cc, tsl * 512:(tsl + 1) * 512],
                    xTr[:, cc, tsl * 512:(tsl + 1) * 512],
                )

        x_slice(0)
        nc.sync.dma_start(wq_sb[:, :, P:HD], wqr[:, :, P:HD])
        nc.gpsimd.dma_start(wk_sb[:, :, P:HD], wkr[:, :, P:HD])
        nc.scalar.dma_start(wv_sb[:], wv.rearrange("(o p) n -> p o n", p=P))
        x_slice(1)
        nc.gpsimd.dma_start(wp_sb[:], wp.rearrange("(o p) n -> p o n", p=P))
        x_slice(2)
        x_slice(3)

        with (
            tc.tile_pool(name="exp", bufs=3) as exp_pool,
            tc.tile_pool(name="accp", bufs=4) as acc_pool,
            tc.tile_pool(name="denp", bufs=2) as den_pool,
            tc.tile_pool(name="otp", bufs=3) as ot_pool,
            tc.tile_pool(name="ps_s", bufs=2, space="PSUM") as ps_s,
            tc.tile_pool(name="ps_av", bufs=2, space="PSUM") as ps_av,
            tc.tile_pool(name="ps_a", bufs=2, space="PSUM") as ps_a,
        ):
            # ---------------- HAM warmup ----------------
            # ~24 scratch matmuls keep the PE busy from t=0 while input DMA
            # streams in, so the HAM clock gate reaches 8/8 before real work
            # arrives (cold PE runs at half clock).
            wu_ps = ps_a.tile([P, 512], f32, tag="a", name="wups")
            for _ in range(24):
                nc.tensor.matmul(
                    wu_ps[:], scr_sb[:, 0:P], scr_sb[:], start=True, stop=True
                )

            # ---------------- qkv emission closures ----------------
            def qkv_group_qk(W_s, B_s, dest, co, tsl):
                def run():
                    ps = ps_a.tile([P, 512], f32, tag="a", name="psqk")
                    for cc in range(CC):
                        nc.tensor.matmul(
                            ps[:],
                            W_s[:, cc, co * P:(co + 1) * P],
                            xT_sb[:, cc, tsl * 512:(tsl + 1) * 512],
                            start=(cc == 0),
                            stop=(cc == CC - 1),
                        )
                    # drain on ScalarE (free per-partition bias add); DVE is
                    # the busier engine in the fused region
                    nc.scalar.activation(
                        dest[:, co, tsl * 512:(tsl + 1) * 512],
                        ps[:],
                        ACTF.Identity,
                        bias=B_s[:, co:co + 1],
                    )
                return run

            def qkv_group_v(tj):
                def run():
                    ps = ps_a.tile([P, 512], f32, tag="a", name="psv")
                    for cc in range(CC):
                        nc.tensor.matmul(
                            ps[:, 0:HD],
                            xT_sb[:, cc, tj * P:(tj + 1) * P],
                            wv_sb[:, cc, :],
                            start=(cc == 0),
                            stop=(cc == CC - 1),
                        )
                    psv = ps[:, 0:HD].rearrange("p (hp hi d) -> p hp hi d", hi=2, d=D)
                    nc.vector.tensor_tensor(
                        v_sb[:, tj, :, :, :], psv, bv_sb[:], ALU.add
                    )
                return run

            def qkv_closures(tsl):
                cl = []
                for co in range(2):
                    cl.append(qkv_group_qk(wq_sb, bq_sb, qT_sb, co, tsl))
                    cl.append(qkv_group_qk(wk_sb, bk_sb, kT_sb, co, tsl))
                for tj in range(4 * tsl, 4 * tsl + 4):
                    cl.append(qkv_group_v(tj))
                return cl

            # ---------------- attention emission closures ----------------
            pending = []  # lagged norms: (ci, hp, av, acc_e, acc_o)

            def emit_norm(ci, hp, av, acc_e, acc_o):
                i0 = ci * 512
                den = ps_a.tile([P, 512], f32, tag="a", name="den")
                for acc, st in ((acc_e, True), (acc_o, False)):
                    for hi in range(2):
                        nc.tensor.matmul(
                            den[hi * D:(hi + 1) * D, :],
                            ones_sb[:],
                            acc[:, hi, :],
                            start=st,
                            stop=not st,
                            skip_group_check=True,
                        )
                rec = den_pool.tile([P, 512], f32, tag="den", name="rec")
                nc.vector.reciprocal_approx_fast(out=rec[:], in_=den[:])
                nc.vector.tensor_tensor(
                    yT_sb[:, hp, i0:i0 + 512], av[:], rec[:], ALU.mult
                )

            def proj_tile(tj):
                def run():
                    ot = ot_pool.tile([P, C], bf16, tag="ot", name="ot")
                    for co in range(2):
                        pps = ps_a.tile([P, 512], f32, tag="a", name="ppsp")
                        for dc in range(2):
                            nc.tensor.matmul(
                                pps[:],
                                yT_sb[:, dc, tj * P:(tj + 1) * P],
                                wp_sb[:, dc, co * 512:(co + 1) * 512],
                                start=(dc == 0),
                                stop=(dc == 1),
                            )
                        nc.vector.tensor_copy(ot[:, co * 512:(co + 1) * 512], pps[:])
                    nc.gpsimd.dma_start(out[tj * P:(tj + 1) * P, :], ot[:])
                return run

            def attn_closures(ci):
                i0 = ci * 512
                njc = 4 * (ci + 1)
                cl = []
                for hp in range(2):
                    av = ps_av.tile([P, 512], f32, tag="av", name="av")
                    acc_e = acc_pool.tile([P, 2, 512], bf16, tag="acc", name="acce")
                    acc_o = acc_pool.tile([P, 2, 512], bf16, tag="acc", name="acco")
                    exs = {}

                    def s_step(jc, hp=hp, exs=exs):
                        def run():
                            diag = jc >= 4 * ci
                            o = (jc - 4 * ci) if diag else 0
                            c0 = o * P
                            sps = ps_s.tile([P, 2, 512], f32, tag="s", name="sps")
                            for hi in range(2):
                                bp = D * hi
                                nc.tensor.matmul(
                                    sps[:, hi, c0:512],
                                    kT_sb[bp:bp + D, hp, jc * P:(jc + 1) * P],
                                    qT_sb[bp:bp + D, hp, i0 + c0:i0 + 512],
                                    start=True,
                                    stop=True,
                                    skip_group_check=True,
                                )
                            ex = exp_pool.tile([P, 2, 512], bf16, tag="ex", name="ex")
                            nc.scalar.activation(
                                ex[:, :, c0:512],
                                sps[:, :, c0:512],
                                ACTF.Exp,
                                scale=float(D) ** -0.5,
                            )
                            if diag:
                                nc.vector.tensor_tensor(
                                    ex[:, :, c0:c0 + P],
                                    ex[:, :, c0:c0 + P],
                                    tri_sb[:, None, :].to_broadcast([P, 2, P]),
                                    ALU.mult,
                                )
                            exs[jc] = (ex, c0)
                        return run

                    def av_step(jc, hp=hp, av=av, acc_e=acc_e, acc_o=acc_o, exs=exs, ci=ci, njc=njc):
                        def run():
                            ex, c0 = exs.pop(jc)
                            for hi in range(2):
                                nc.tensor.matmul(
                                    av[hi * D:(hi + 1) * D, c0:512],
                                    v_sb[:, jc, hp, hi, :],
                                    ex[:, hi, c0:512],
                                    start=(jc == 0),
                                    stop=(jc == njc - 1),
                                    skip_group_check=True,
                                )
                            acc = acc_e if jc % 2 == 0 else acc_o
                            if jc < 2:
                                # first member of each chain: copy (+ zero the
                                # columns this diag chunk doesn't cover)
                                if c0 > 0:
                                    nc.vector.memset(acc[:, :, 0:c0], 0.0)
                                nc.vector.tensor_copy(
                                    acc[:, :, c0:512], ex[:, :, c0:512]
                                )
                            else:
                                nc.vector.tensor_tensor(
                                    acc[:, :, c0:512],
                                    acc[:, :, c0:512],
                                    ex[:, :, c0:512],
                                    ALU.add,
                                )
                        return run

                    # software-pipelined: S one step ahead of AV
                    cl.append(s_step(0))
                    for jc in range(1, njc):
                        cl.append(s_step(jc))
                        cl.append(av_step(jc - 1))
                    cl.append(av_step(njc - 1))

                    def push_norm(ci=ci, hp=hp, av=av, acc_e=acc_e, acc_o=acc_o):
                        def run():
                            pending.append((ci, hp, av, acc_e, acc_o))
                            if len(pending) > 1:
                                emit_norm(*pending.pop(0))
                        return run

                    cl.append(push_norm())
                if ci >= 1:
                    for tj in range(4 * (ci - 1), 4 * (ci - 1) + 4):
                        cl.append(proj_tile(tj))
                return cl

            # ---------------- fused master schedule ----------------
            for tsl in range(IC + 1):
                qg = qkv_closures(tsl) if tsl < IC else []
                at = attn_closures(tsl - 1) if tsl >= 1 else []
                n, m = len(qg), len(at)
                if n == 0:
                    for c in at:
                        c()
                else:
                    k = 0
                    for i, g in enumerate(qg):
                        g()
                        k2 = (i + 1) * m // n
                        for c in at[k:k2]:
                            c()
                        k = k2
            while pending:
                emit_norm(*pending.pop(0))
            for tj in range(4 * (IC - 1), 4 * (IC - 1) + 4):
                proj_tile(tj)()
    nc.compile()
    return nc


def _get_nc():
    global _NC
    if _NC is None:
        _NC = _build_nc()
    return _NC


def _make_in_maps(x, W_qkv, b_qkv, W_proj):
    import ml_dtypes

    bf = ml_dtypes.bfloat16
    tri = np.ascontiguousarray(np.triu(np.ones((P, P), dtype=np.float32)).astype(bf))
    onesd = np.ones((P, D), dtype=bf)
    in_maps = []
    for c in range(8):
        b, g = divmod(c, 4)
        s = slice(HD * g, HD * g + HD)
        sk = slice(C + HD * g, C + HD * g + HD)
        sv = slice(2 * C + HD * g, 2 * C + HD * g + HD)
        in_maps.append({
            "xT": np.ascontiguousarray(x[b].T.astype(bf)),
            "wq": np.ascontiguousarray(W_qkv[:, s].astype(bf)),
            "wk": np.ascontiguousarray(W_qkv[:, sk].astype(bf)),
            "wv": np.ascontiguousarray(W_qkv[:, sv].astype(bf)),
            "bq": np.ascontiguousarray(b_qkv[s].reshape(2, P).T),
            "bk": np.ascontiguousarray(b_qkv[sk].reshape(2, P).T),
            "bv": np.ascontiguousarray(np.broadcast_to(b_qkv[sv], (P, HD))),
            "wp": np.ascontiguousarray(W_proj[s, :].astype(bf)),
            "tri": tri,
            "onesd": onesd,
        })
    return in_maps


def kernel(x, W_qkv, b_qkv, W_proj, b_proj):
    global LAST_RESULTS
    from concourse import bass_utils

    x = np.asarray(x, dtype=np.float32)
    W_qkv = np.asarray(W_qkv, dtype=np.float32)
    b_qkv = np.asarray(b_qkv, dtype=np.float32)
    W_proj = np.asarray(W_proj, dtype=np.float32)
    b_proj = np.asarray(b_proj, dtype=np.float32)

    nc = _get_nc()
    in_maps = _make_in_maps(x, W_qkv, b_qkv, W_proj)
    res = bass_utils.run_bass_kernel_spmd(nc, in_maps, core_ids=list(range(8)))
    LAST_RESULTS = res
    ys = []
    for b in range(2):
        y = res.results[4 * b]["out"].astype(np.float32)
        for g in range(1, 4):
            y = y + res.results[4 * b + g]["out"].astype(np.float32)
        ys.append(y + b_proj)
    return np.stack(ys, axis=0)


# revision 15
# speedup vs baseline: 1.0284x; 1.0284x over previous
"""Causal self-attention (B=2, T=2048, C=1024, H=16) on 8 Trainium2 cores.

Sharding: data-parallel over batch (2) x tensor-parallel over heads (4 groups
of 4 heads). Core c handles batch b = c//4, head group g = c%4 (heads 4g..4g+3).
Each core computes its qkv column slice, full causal TxT attention for its 4
heads, and a partial row-parallel projection. Host sums the 4 partial proj
outputs per batch and adds b_proj.

v2 design notes (vs the f32r v1):
- everything bf16 end-to-end (inputs, SBUF intermediates, output); PSUM stays
  fp32. Halves DMA traffic (6.5MB in / 4MB out per core) and enables the PE's
  fast-weight-load path (2x LDWEIGHTS for non-fp32 128-col stationaries).
- the scalar engine's exp is the true serializer (~10.5M score exps/core at
  1 elem/lane/cycle): the qkv projection phase is fused INTO the attention
  loop (qkv for t-slice tsl emits interleaved with attention for i-block
  tsl-1) so ACT starts exp'ing ~10us in and stays saturated while the PE
  retires qkv/score/AV/proj matmuls underneath it.
- AV matmuls are column-tiled pairs (head hi=0 -> PE cols 0:64, hi=1 ->
  64:128) running concurrently, each M=64 with no zero padding; the two
  score matmuls of a head pair stay row-tiled (K=64 quadrants) as in v1.
- softmax denominators: DVE accumulates masked exp tiles into two bf16
  chains (even/odd j-chunks); a column-tiled pair of K=128 ones-stationary
  matmuls then both reduces over j and broadcasts den across partitions in
  PSUM rows matching the AV layout; gpsimd drains den to SBUF and one DVE
  divide normalizes into yT. (No ACT Reciprocal: it lives in a different
  ACT table set than Exp and each switch costs ~2.7us.)
- PSUM budget exactly 8 banks: scores [128,2,512]x2 (4) + AV [128,512]x2 (2)
  + one shared round-robin tag for qkv/den/proj psum [128,512]x2 (2).
- proj output copies + DMA and den drains ride the otherwise-idle gpsimd.
"""

import os
import sys

sys.path.insert(0, "/opt/trn_rl_repo")

import numpy as np

P = 128
T = 2048
C = 1024
D = 64
HPC = 4          # heads per core
HD = HPC * D     # 256 qkv columns per core
CC = C // P      # 8 contraction chunks
TC = T // P      # 16 t-chunks of 128
IC = T // 512    # 4 i-chunks of 512

_NC = None
LAST_RESULTS = None


def _build_nc():
    import concourse.mybir as mybir
    import concourse.tile as tile
    from concourse import bacc
    from contextlib import ExitStack

    dt = mybir.dt
    f32 = dt.float32
    bf16 = dt.bfloat16
    ALU = mybir.AluOpType
    ACTF = mybir.ActivationFunctionType

    nc = bacc.Bacc(
        "TRN2",
        target_bir_lowering=False,
        debug=False,
        enable_asserts=False,
        num_devices=8,
    )

    # host pre-layouts so every DMA line is >=4KB contiguous per partition:
    # xh[p, tsl, cc, s] = x.T[cc*128+p, tsl*512+s]; w*[p, cc, n]; wp[p, dc, n]
    xh = nc.dram_tensor("xh", [P, IC, CC, 512], bf16, kind="ExternalInput").ap()
    wq = nc.dram_tensor("wq", [P, CC, HD], bf16, kind="ExternalInput").ap()
    wk = nc.dram_tensor("wk", [P, CC, HD], bf16, kind="ExternalInput").ap()
    wv = nc.dram_tensor("wv", [P, CC, HD], bf16, kind="ExternalInput").ap()
    bq = nc.dram_tensor("bq", [P, 2], f32, kind="ExternalInput").ap()
    bk = nc.dram_tensor("bk", [P, 2], f32, kind="ExternalInput").ap()
    bv = nc.dram_tensor("bv", [P, HD], f32, kind="ExternalInput").ap()
    wp = nc.dram_tensor("wp", [P, 2, C], bf16, kind="ExternalInput").ap()
    tri = nc.dram_tensor("tri", [P, P], bf16, kind="ExternalInput").ap()
    onesd = nc.dram_tensor("onesd", [P, D], bf16, kind="ExternalInput").ap()
    out = nc.dram_tensor("out", [T, C], bf16, kind="ExternalOutput").ap()

    with tile.TileContext(nc) as tc, ExitStack() as ctx:
        persist = ctx.enter_context(tc.tile_pool(name="persist", bufs=1))
        xT_sb = persist.tile([P, CC, T], bf16, name="xTs")
        wq_sb = persist.tile([P, CC, HD], bf16, name="wqs")
        wk_sb = persist.tile([P, CC, HD], bf16, name="wks")
        wv_sb = persist.tile([P, CC, HD], bf16, name="wvs")
        wp_sb = persist.tile([P, 2, C], bf16, name="wps")
        qT_sb = persist.tile([P, 2, T], bf16, name="qT")    # [d%128, hp, t]
        kT_sb = persist.tile([P, 2, T], bf16, name="kT")
        v_sb = persist.tile([P, TC, 2, 2, D], bf16, name="v")  # [t%128, tc, hp, hi, d]
        yT_sb = persist.tile([P, 2, T], bf16, name="yT")
        tri_sb = persist.tile([P, P], bf16, name="tris")
        ones_sb = persist.tile([P, D], bf16, name="ones")
        bq_sb = persist.tile([P, 2], f32, name="bqs")
        bk_sb = persist.tile([P, 2], f32, name="bks")
        bv_sb = persist.tile([P, 2, 2, D], f32, name="bvs")
        scr_sb = persist.tile([P, 512], bf16, name="scr")

        # ---- input DMA schedule: whole-tensor weight loads + x in 512KB
        # (tsl, cc-half) slices, all >=4KB contiguous per partition line,
        # spread over the three DMA-capable engine queues ----
        nc.gpsimd.memset(scr_sb[:], 0.0)
        nc.sync.dma_start(wq_sb[:], wq)
        nc.gpsimd.dma_start(wk_sb[:], wk)
        nc.scalar.dma_start(tri_sb[:], tri)
        nc.scalar.dma_start(bq_sb[:], bq)
        nc.scalar.dma_start(bk_sb[:], bk)
        nc.scalar.dma_start(ones_sb[:], onesd)
        nc.scalar.dma_start(
            bv_sb[:], bv.rearrange("p (hp hi d) -> p hp hi d", hi=2, d=D)
        )
        nc.scalar.dma_start(wv_sb[:], wv)

        def x_slice(tsl):
            for h, eng in ((0, nc.sync), (1, nc.gpsimd)):
                eng.dma_start(
                    xT_sb[:, 4 * h:4 * h + 4, tsl * 512:(tsl + 1) * 512],
                    xh[:, tsl, 4 * h:4 * h + 4, :],
                )

        x_slice(0)
        x_slice(1)
        nc.scalar.dma_start(wp_sb[:], wp)
        x_slice(2)
        x_slice(3)

        with (
            tc.tile_pool(name="exp", bufs=3) as exp_pool,
            tc.tile_pool(name="accp", bufs=4) as acc_pool,
            tc.tile_pool(name="denp", bufs=2) as den_pool,
            tc.tile_pool(name="otp", bufs=3) as ot_pool,
            tc.tile_pool(name="ps_s", bufs=2, space="PSUM") as ps_s,
            tc.tile_pool(name="ps_av", bufs=2, space="PSUM") as ps_av,
            tc.tile_pool(name="ps_a", bufs=2, space="PSUM") as ps_a,
        ):
            # ---------------- HAM warmup ----------------
            # ~24 scratch matmuls keep the PE busy from t=0 while input DMA
            # streams in, so the HAM clock gate reaches 8/8 before real work
            # arrives (cold PE runs at half clock). scr_sb is deliberately
            # uninitialized (no dependency -> starts right at engine boot);
            # the warmup psum is never read.
            wu_ps = ps_a.tile([P, 512], f32, tag="a", name="wups")
            for _ in range(24):
                nc.tensor.matmul(
                    wu_ps[:], scr_sb[:, 0:P], scr_sb[:], start=True, stop=True
                )

            # ---------------- qkv emission closures ----------------
            def qkv_group_qk(W_s, B_s, dest, co, tsl):
                def run():
                    ps = ps_a.tile([P, 512], f32, tag="a", name="psqk")
                    for cc in range(CC):
                        nc.tensor.matmul(
                            ps[:],
                            W_s[:, cc, co * P:(co + 1) * P],
                            xT_sb[:, cc, tsl * 512:(tsl + 1) * 512],
                            start=(cc == 0),
                            stop=(cc == CC - 1),
                        )
                    # drain on ScalarE (free per-partition bias add); DVE is
                    # the busier engine in the fused region
                    nc.scalar.activation(
                        dest[:, co, tsl * 512:(tsl + 1) * 512],
                        ps[:],
                        ACTF.Identity,
                        bias=B_s[:, co:co + 1],
                    )
                return run

            def qkv_group_v(tj):
                def run():
                    ps = ps_a.tile([P, 512], f32, tag="a", name="psv")
                    for cc in range(CC):
                        nc.tensor.matmul(
                            ps[:, 0:HD],
                            xT_sb[:, cc, tj * P:(tj + 1) * P],
                            wv_sb[:, cc, :],
                            start=(cc == 0),
                            stop=(cc == CC - 1),
                        )
                    psv = ps[:, 0:HD].rearrange("p (hp hi d) -> p hp hi d", hi=2, d=D)
                    nc.vector.tensor_tensor(
                        v_sb[:, tj, :, :, :], psv, bv_sb[:], ALU.add
                    )
                return run

            def qkv_closures(tsl):
                cl = []
                for co in range(2):
                    cl.append(qkv_group_qk(wq_sb, bq_sb, qT_sb, co, tsl))
                    cl.append(qkv_group_qk(wk_sb, bk_sb, kT_sb, co, tsl))
                for tj in range(4 * tsl, 4 * tsl + 4):
                    cl.append(qkv_group_v(tj))
                return cl

            # ---------------- attention emission closures ----------------
            pending = []  # lagged norms: (ci, hp, av, acc_e, acc_o)

            def emit_norm(ci, hp, av, acc_e, acc_o):
                i0 = ci * 512
                den = ps_a.tile([P, 512], f32, tag="a", name="den")
                for acc, st in ((acc_e, True), (acc_o, False)):
                    for hi in range(2):
                        nc.tensor.matmul(
                            den[hi * D:(hi + 1) * D, :],
                            ones_sb[:],
                            acc[:, hi, :],
                            start=st,
                            stop=not st,
                            skip_group_check=True,
                        )
                rec = den_pool.tile([P, 512], f32, tag="den", name="rec")
                nc.vector.reciprocal_approx_fast(out=rec[:], in_=den[:])
                nc.vector.tensor_tensor(
                    yT_sb[:, hp, i0:i0 + 512], av[:], rec[:], ALU.mult
                )

            def proj_tile(tj):
                def run():
                    ot = ot_pool.tile([P, C], bf16, tag="ot", name="ot")
                    for co in range(2):
                        pps = ps_a.tile([P, 512], f32, tag="a", name="ppsp")
                        for dc in range(2):
                            nc.tensor.matmul(
                                pps[:],
                                yT_sb[:, dc, tj * P:(tj + 1) * P],
                                wp_sb[:, dc, co * 512:(co + 1) * 512],
                                start=(dc == 0),
                                stop=(dc == 1),
                            )
                        nc.vector.tensor_copy(ot[:, co * 512:(co + 1) * 512], pps[:])
                    nc.gpsimd.dma_start(out[tj * P:(tj + 1) * P, :], ot[:])
                return run

            def attn_closures(ci):
                i0 = ci * 512
                njc = 4 * (ci + 1)
                cl = []
                for hp in range(2):
                    av = ps_av.tile([P, 512], f32, tag="av", name="av")
                    acc_e = acc_pool.tile([P, 2, 512], bf16, tag="acc", name="acce")
                    acc_o = acc_pool.tile([P, 2, 512], bf16, tag="acc", name="acco")
                    exs = {}

                    def s_step(jc, hp=hp, exs=exs):
                        def run():
                            diag = jc >= 4 * ci
                            o = (jc - 4 * ci) if diag else 0
                            c0 = o * P
                            sps = ps_s.tile([P, 2, 512], f32, tag="s", name="sps")
                            for hi in range(2):
                                bp = D * hi
                                nc.tensor.matmul(
                                    sps[:, hi, c0:512],
                                    kT_sb[bp:bp + D, hp, jc * P:(jc + 1) * P],
                                    qT_sb[bp:bp + D, hp, i0 + c0:i0 + 512],
                                    start=True,
                                    stop=True,
                                    skip_group_check=True,
                                )
                            ex = exp_pool.tile([P, 2, 512], bf16, tag="ex", name="ex")
                            nc.scalar.activation(
                                ex[:, :, c0:512],
                                sps[:, :, c0:512],
                                ACTF.Exp,
                                scale=float(D) ** -0.5,
                            )
                            if diag:
                                nc.vector.tensor_tensor(
                                    ex[:, :, c0:c0 + P],
                                    ex[:, :, c0:c0 + P],
                                    tri_sb[:, None, :].to_broadcast([P, 2, P]),
                                    ALU.mult,
                                )
                            exs[jc] = (ex, c0)
                        return run

                    def av_step(jc, hp=hp, av=av, acc_e=acc_e, acc_o=acc_o, exs=exs, ci=ci, njc=njc):
                        def run():
                            ex, c0 = exs.pop(jc)
                            for hi in range(2):
                                nc.tensor.matmul(
                                    av[hi * D:(hi + 1) * D, c0:512],
                                    v_sb[:, jc, hp, hi, :],
                                    ex[:, hi, c0:512],
                                    start=(jc == 0),
                                    stop=(jc == njc - 1),
                                    skip_group_check=True,
                                )
                            acc = acc_e if jc % 2 == 0 else acc_o
                            if jc < 2:
                                # first member of each chain: copy (+ zero the
                                # columns this diag chunk doesn't cover)
                                if c0 > 0:
                                    nc.vector.memset(acc[:, :, 0:c0], 0.0)
                                nc.vector.tensor_copy(
                                    acc[:, :, c0:512], ex[:, :, c0:512]
                                )
                            else:
                                nc.vector.tensor_tensor(
                                    acc[:, :, c0:512],
                                    acc[:, :, c0:512],
                                    ex[:, :, c0:512],
                                    ALU.add,
                                )
                        return run

                    # software-pipelined: S one step ahead of AV
                    cl.append(s_step(0))
                    for jc in range(1, njc):
                        cl.append(s_step(jc))
                        cl.append(av_step(jc - 1))
                    cl.append(av_step(njc - 1))

                    def push_norm(ci=ci, hp=hp, av=av, acc_e=acc_e, acc_o=acc_o):
                        def run():
                            pending.append((ci, hp, av, acc_e, acc_o))
                            if len(pending) > 1:
                                emit_norm(*pending.pop(0))
                        return run

                    cl.append(push_norm())
                if ci >= 1:
                    for tj in range(4 * (ci - 1), 4 * (ci - 1) + 4):
                        cl.append(proj_tile(tj))
                return cl

            # ---------------- fused master schedule ----------------
            for tsl in range(IC + 1):
                qg = qkv_closures(tsl) if tsl < IC else []
                at = attn_closures(tsl - 1) if tsl >= 1 else []
                n, m = len(qg), len(at)
                if n == 0:
                    for c in at:
                        c()
                else:
                    k = 0
                    for i, g in enumerate(qg):
                        g()
                        k2 = (i + 1) * m // n
                        for c in at[k:k2]:
                            c()
                        k = k2
            while pending:
                emit_norm(*pending.pop(0))
            for tj in range(4 * (IC - 1), 4 * (IC - 1) + 4):
                proj_tile(tj)()
    nc.compile()
    return nc


def _get_nc():
    global _NC
    if _NC is None:
        _NC = _build_nc()
    return _NC


def _make_in_maps(x, W_qkv, b_qkv, W_proj):
    import ml_dtypes

    bf = ml_dtypes.bfloat16
    tri = np.ascontiguousarray(np.triu(np.ones((P, P), dtype=np.float32)).astype(bf))
    onesd = np.ones((P, D), dtype=bf)
    in_maps = []
    for c in range(8):
        b, g = divmod(c, 4)
        s = slice(HD * g, HD * g + HD)
        sk = slice(C + HD * g, C + HD * g + HD)
        sv = slice(2 * C + HD * g, 2 * C + HD * g + HD)
        xTb = x[b].T.astype(bf)  # [C, T]
        in_maps.append({
            "xh": np.ascontiguousarray(
                xTb.reshape(CC, P, IC, 512).transpose(1, 2, 0, 3)
            ),
            "wq": np.ascontiguousarray(
                W_qkv[:, s].astype(bf).reshape(CC, P, HD).transpose(1, 0, 2)
            ),
            "wk": np.ascontiguousarray(
                W_qkv[:, sk].astype(bf).reshape(CC, P, HD).transpose(1, 0, 2)
            ),
            "wv": np.ascontiguousarray(
                W_qkv[:, sv].astype(bf).reshape(CC, P, HD).transpose(1, 0, 2)
            ),
            "bq": np.ascontiguousarray(b_qkv[s].reshape(2, P).T),
            "bk": np.ascontiguousarray(b_qkv[sk].reshape(2, P).T),
            "bv": np.ascontiguousarray(np.broadcast_to(b_qkv[sv], (P, HD))),
            "wp": np.ascontiguousarray(
                W_proj[s, :].astype(bf).reshape(2, P, C).transpose(1, 0, 2)
            ),
            "tri": tri,
            "onesd": onesd,
        })
    return in_maps


def kernel(x, W_qkv, b_qkv, W_proj, b_proj):
    global LAST_RESULTS
    from concourse import bass_utils

    x = np.asarray(x, dtype=np.float32)
    W_qkv = np.asarray(W_qkv, dtype=np.float32)
    b_qkv = np.asarray(b_qkv, dtype=np.float32)
    W_proj = np.asarray(W_proj, dtype=np.float32)
    b_proj = np.asarray(b_proj, dtype=np.float32)

    nc = _get_nc()
    in_maps = _make_in_maps(x, W_qkv, b_qkv, W_proj)
    res = bass_utils.run_bass_kernel_spmd(nc, in_maps, core_ids=list(range(8)))
    LAST_RESULTS = res
    ys = []
    for b in range(2):
        y = res.results[4 * b]["out"].astype(np.float32)
        for g in range(1, 4):
            y = y + res.results[4 * b + g]["out"].astype(np.float32)
        ys.append(y + b_proj)
    return np.stack(ys, axis=0)
